# revision 20
# baseline (speedup 1.0000x reference)
"""Trainium2 Bass kernel for a dense transformer block (B=2, S=2048, D=1024,
H=16, HS=64, DFF=4096) on 8 NeuronCores.

Sharding: data-parallel, zero collectives. Core c = (b, i) with b = c // 4,
i = c % 4 handles batch b and the 512 query tokens made of two 256-chunks
{i, 7-i} (chunk j covers tokens [256j, 256j+256)). Pairing chunk i (causal
keyspan <= 1024) with chunk 7-i (keyspan <= 2048) makes the per-core
attention work uniform, so all 8 cores run the same instruction stream
(SPMD) with different data. K/V are computed redundantly for the full 2048
tokens of the core's batch (cheaper than an AllGather on this runtime).

Layout: feature-major activations (x^T [D, T]) everywhere, so every linear
layer is matmul(lhsT=W[d_in, d_out] tile, rhs=act^T) with no transposes.
V is produced token-major directly by swapping matmul operand roles.
Matmuls run as float32r (FP22-truncated fp32) which is full-rate on the PE
at moving free dim >= 256. LayerNorm affine (gamma/beta) and the 1/sqrt(HS)
score scale are folded into the weights on the host. Causal masking is done
on-device by comparing a per-core query-position threshold against the key
index, fused into one vector op per score block. The softmax denominator
rides along the att@V matmul as an extra block of ones columns in lhsT.
"""
import math
from contextlib import ExitStack

import numpy as np

B, S, D = 2, 2048, 1024
H, HS = 16, 64
DFF = 4 * D
EPS = 1e-5
N_CORES = 8
P = 128
QC = 256  # query chunk width
NQ = 2 * QC  # queries per core
KSPAN = (1024, 2048)  # keyspan for query chunk A (i) / B (7-i)
KD = D // P  # 8 contraction chunks of 128
TB = 512  # token block for LN1/QKV
NTB = S // TB  # 4
DHALF = DFF // 2 // P  # 16 dff chunks per half
DEBUG = False


def _build_nc():
    import concourse.tile as tile
    from concourse import bacc, mybir

    F32 = mybir.dt.float32
    BF16 = mybir.dt.bfloat16
    FP16 = mybir.dt.float16
    AF = mybir.ActivationFunctionType
    ALU = mybir.AluOpType

    def r(ap):
        return ap

    nc = bacc.Bacc(
        "TRN2", target_bir_lowering=False, debug=False, num_devices=N_CORES
    )

    _ctr = [0]

    def T(pool, shape, tag, dt=None):
        _ctr[0] += 1
        return pool.tile(shape, dt or F32, tag=tag, name=f"{tag}_{_ctr[0]}")

    def din(name, shape, dt=None):
        return nc.dram_tensor(
            name, shape, dt or F32, kind="ExternalInput"
        ).ap()

    xt = din("xt", [D, S], BF16)  # x^T for this core's batch
    xq = din("xq", [D, NQ])  # x^T own tokens, fp32 (residual path)
    xqb = din("xqb", [D, NQ], BF16)  # x^T own tokens, bf16 (LN path)
    # weights are host-prearranged so every DMA lands as 2KB+ runs:
    # streamed weights as [m-block, p, kd, c]; wk resident as [p, kd, c]
    wq = din("wq", [KD, P, KD, P], BF16)  # folded gamma*Wq/sqrt(HS)
    wk = din("wk", [P, KD, D], BF16)
    wv = din("wv", [2, P, KD, TB], BF16)
    wp = din("wp", [KD, P, KD, P], BF16)
    w1 = din("w1", [DFF // P, P, KD, P], BF16)
    w2 = din("w2", [2, KD, P, DHALF, P], BF16)
    bqv = din("bq", [D])
    bkv = din("bk", [D])
    bvv = din("bv", [D])
    bpv = din("bp", [D])
    b1v = din("b1", [DFF])
    b2v = din("b2", [D])
    # fp16 mask thresholds (integers <= 2048 are exact in fp16):
    # mthr[0, j, :] = [A-chunk thr | B-chunk thr] for both hh of a pair;
    # mthr2[0, j, :] = [B thr | B thr] (only cols 0:QC read when nq=QC)
    mthr = din("mthr", [1, 2, NQ])
    mthr2 = din("mthr2", [1, 2, NQ])
    kidx = din("kidx", [P, S // P], FP16)  # kidx[p, kt] = kt*128 + p
    out = nc.dram_tensor("out", [D, NQ], F32, kind="ExternalOutput").ap()
    if DEBUG:
        dbg = {
            n: nc.dram_tensor(n, sh, dt, kind="ExternalOutput").ap()
            for n, sh, dt in [
                ("d_h", [D, S], BF16),
                ("d_hq", [D, NQ], BF16),
                ("d_qt", [D, NQ], BF16),
                ("d_kt", [D, S], BF16),
                ("d_v", [S, D], BF16),
                ("d_att", [D, NQ], BF16),
                ("d_res1", [D, NQ], F32),
                ("d_h2", [D, NQ], BF16),
                ("d_thrb", [P, NQ], F32),
                ("d_bvb", [P, D], F32),
            ]
        }
    kt_d = nc.dram_tensor("kt_d", [D, S], BF16).ap()  # K^T staging
    v_d = nc.dram_tensor("v_d", [S, D], BF16).ap()  # V staging

    with tile.TileContext(nc) as tc, ExitStack() as g:
        # ---- global pools ----
        consts = g.enter_context(tc.tile_pool(name="consts", bufs=1))
        small = g.enter_context(tc.tile_pool(name="small", bufs=8))
        bc = g.enter_context(tc.tile_pool(name="bc", bufs=4))

        ones = T(consts, [P, 1], "ones", BF16)
        nc.vector.memset(ones, 1.0)
        ones_row = T(consts, [P, P], "onesr", BF16)
        nc.vector.memset(ones_row, 1.0)
        eps_t = T(consts, [1, 1], "eps")
        nc.vector.memset(eps_t, EPS)
        mthr_sb = T(consts, [1, NQ], "mthr")
        nc.sync.dma_start(mthr_sb, mthr.rearrange("a b c -> a (b c)")[:, :NQ])
        mthr_bf = T(consts, [P, NQ], "mthrb")
        nc.gpsimd.partition_broadcast(mthr_bf, mthr_sb)
        mthr2_sb = T(consts, [1, NQ], "mthr2")
        nc.sync.dma_start(
            mthr2_sb, mthr2.rearrange("a b c -> a (b c)")[:, :NQ]
        )
        mthr2_bf = T(consts, [P, NQ], "mthr2b")
        nc.gpsimd.partition_broadcast(mthr2_bf, mthr2_sb)
        kidx_sb = T(consts, [P, S // P], "kidx", FP16)
        nc.sync.dma_start(kidx_sb, kidx)

        def load_bias(v, n, tag):
            t = T(consts, [P, n // P], tag)
            nc.sync.dma_start(t, v.rearrange("(m p) -> p m", p=P))
            return t

        bq_sb = load_bias(bqv, D, "bq")
        bk_sb = load_bias(bkv, D, "bk")
        bp_sb = load_bias(bpv, D, "bp")
        b1_sb = load_bias(b1v, DFF, "b1")
        b2_sb = load_bias(b2v, D, "b2")
        bv_sb = T(consts, [1, D], "bvrow")
        nc.sync.dma_start(bv_sb, bvv.rearrange("(a d) -> a d", a=1))
        bv_b = T(consts, [P, D], "bvb")
        nc.gpsimd.partition_broadcast(bv_b, bv_sb)

        sqpool = None  # set per-stage; used by ln_stats
        mmps = None  # per-stage matmul psum pool; used by ln_stats too

        def ln_stats(x_tile, n):
            """x_tile [P, KD, n] feature-major -> (inv_b, nmi_b) [P, n]."""
            ps_s = T(mmps, [1, TB], "mm")[:, :n]
            ps_q = T(mmps, [1, TB], "mm")[:, :n]
            for kd in range(KD):
                sq = T(sqpool, [P, TB], "sq", BF16)[:, :n]
                nc.scalar.activation(sq, x_tile[:, kd, :], AF.Square)
                nc.tensor.matmul(
                    ps_s, r(ones), r(x_tile[:, kd, :]),
                    start=(kd == 0), stop=(kd == KD - 1),
                )
                nc.tensor.matmul(
                    ps_q, r(ones), r(sq),
                    start=(kd == 0), stop=(kd == KD - 1),
                )
            mu = T(small, [1, TB], "sm")[:, :n]
            nc.scalar.activation(mu, ps_s, AF.Copy, scale=1.0 / D)
            ex2 = T(small, [1, TB], "sm")[:, :n]
            nc.scalar.activation(ex2, ps_q, AF.Copy, scale=1.0 / D)
            var = T(small, [1, TB], "sm")[:, :n]
            nc.vector.tensor_mul(var, mu, mu)
            nc.vector.tensor_sub(var, ex2, var)
            std = T(small, [1, TB], "sm")[:, :n]
            nc.scalar.activation(std, var, AF.Sqrt, bias=eps_t)
            inv = T(small, [1, TB], "sm")[:, :n]
            nc.vector.reciprocal_approx_fast(inv, std)
            nmi = T(small, [1, TB], "sm")[:, :n]
            nc.vector.tensor_mul(nmi, mu, inv)
            nc.scalar.mul(nmi, nmi, -1.0)
            invh = T(small, [1, TB], "smh", BF16)[:, :n]
            nc.vector.tensor_copy(invh, inv)
            nmih = T(small, [1, TB], "smh", BF16)[:, :n]
            nc.vector.tensor_copy(nmih, nmi)
            inv_b = T(bc, [P, TB], "bc", BF16)[:, :n]
            nc.gpsimd.partition_broadcast(inv_b, invh)
            nmi_b = T(bc, [P, TB], "bc", BF16)[:, :n]
            nc.gpsimd.partition_broadcast(nmi_b, nmih)
            return inv_b, nmi_b

        def ln_apply(dst, x_tile, inv_b, nmi_b):
            for kd in range(KD):
                nc.vector.tensor_mul(dst[:, kd, :], x_tile[:, kd, :], inv_b)
                nc.vector.tensor_add(dst[:, kd, :], dst[:, kd, :], nmi_b)

        respool = g.enter_context(tc.tile_pool(name="respool", bufs=1))
        res1 = T(respool, [P, KD, NQ], "res1")  # x + att (fp32)
        res1b = T(respool, [P, KD, NQ], "res1b", BF16)  # bf16 shadow
        qs = ExitStack()  # qt/xq pools: closed after stage C
        qpool = qs.enter_context(tc.tile_pool(name="qpool", bufs=1))
        xqpool = qs.enter_context(tc.tile_pool(name="xqpool", bufs=1))
        qt_all = T(qpool, [P, KD, NQ], "qt", BF16)  # Q^T
        xq_t = T(xqpool, [P, KD, NQ], "xq")  # x^T own tokens (fp32)
        xqb_t = T(xqpool, [P, KD, NQ], "xqb", BF16)

        # ================= Stage A+B: LN1, QKV =================
        with ExitStack() as sa:
            mmps = sa.enter_context(
                tc.tile_pool(name="mmpsA", bufs=2, space="PSUM")
            )
            xpool = sa.enter_context(tc.tile_pool(name="xpool", bufs=NTB))
            sqpool = sa.enter_context(tc.tile_pool(name="sqpool", bufs=3))
            hqpool = sa.enter_context(tc.tile_pool(name="hqpool", bufs=1))
            wpool = sa.enter_context(tc.tile_pool(name="wpool", bufs=3))
            wkpool = sa.enter_context(tc.tile_pool(name="wkpool", bufs=1))
            vwpool = sa.enter_context(tc.tile_pool(name="vwpool", bufs=2))
            kopool = sa.enter_context(tc.tile_pool(name="kopool", bufs=3))

            # weights prefetched on the gpsimd (SWDGE) queue so they
            # never delay the x-block loads on the sync queue
            wkf = T(wkpool, [P, KD, D], "wkf", BF16)
            nc.gpsimd.dma_start(wkf, wk)
            wvb = []
            for n in range(2):
                wvt = T(vwpool, [P, KD, TB], "wv", BF16)
                nc.gpsimd.dma_start(wvt, wv[n])
                wvb.append(wvt)

            # x blocks first on the sync queue (LN -> K chain needs them)
            x_blk = []
            for tb in range(NTB):
                x_t = T(xpool, [P, KD, TB], "x", BF16)
                cols = slice(tb * TB, (tb + 1) * TB)
                nc.sync.dma_start(
                    x_t, xt[:, cols].rearrange("(kd p) t -> p kd t", p=P)
                )
                x_blk.append(x_t)
            nc.sync.dma_start(xqb_t, xqb.rearrange("(kd p) t -> p kd t", p=P))

            # per block: LN (in place: x tile becomes h tile) then its
            # K^T columns immediately, so the PE fills from the start
            h_blk = []
            for n in range(NTB):
                x_t = x_blk[n]
                inv_b, nmi_b = ln_stats(x_t, TB)
                ln_apply(x_t, x_t, inv_b, nmi_b)
                h_blk.append(x_t)
                for m in range(KD):
                    ps = T(mmps, [P, TB], "mm")
                    for kd in range(KD):
                        nc.tensor.matmul(
                            ps,
                            r(wkf[:, kd, m * P : (m + 1) * P]),
                            r(x_t[:, kd, :]),
                            start=(kd == 0), stop=(kd == KD - 1),
                        )
                    ko = T(kopool, [P, TB], "ko", BF16)
                    nc.scalar.activation(
                        ko, ps, AF.Identity, bias=bk_sb[:, m : m + 1]
                    )
                    nc.sync.dma_start(
                        kt_d[m * P : (m + 1) * P, n * TB : (n + 1) * TB], ko
                    )

            # LN for this core's own 512 query tokens (recomputed so the
            # program stays uniform across cores), then Q^T
            hq = T(hqpool, [P, KD, NQ], "hq", BF16)
            inv_b, nmi_b = ln_stats(xqb_t, NQ)
            ln_apply(hq, xqb_t, inv_b, nmi_b)

            for m in range(KD):
                wblk = T(wpool, [P, KD, P], "w", BF16)
                nc.gpsimd.dma_start(wblk, wq[m])
                ps = T(mmps, [P, TB], "mm")[:, :NQ]
                for kd in range(KD):
                    nc.tensor.matmul(
                        ps, r(wblk[:, kd, :]), r(hq[:, kd, :]),
                        start=(kd == 0), stop=(kd == KD - 1),
                    )
                nc.scalar.activation(
                    qt_all[:, m, :], ps, AF.Identity, bias=bq_sb[:, m : m + 1]
                )

            # residual-path copy of x^T (fp32) — only needed at proj time
            nc.sync.dma_start(xq_t, xq.rearrange("(kd p) t -> p kd t", p=P))

            # V token-major: lhsT = h tile (stationary), rhs = wv (moving)
            for n in range(2):
                for t in range(S // P):
                    tb, toff = t // (TB // P), (t % (TB // P)) * P
                    ps = T(mmps, [P, TB], "mm")
                    for kd in range(KD):
                        nc.tensor.matmul(
                            ps,
                            r(h_blk[tb][:, kd, toff : toff + P]),
                            r(wvb[n][:, kd, :]),
                            start=(kd == 0), stop=(kd == KD - 1),
                        )
                    vo = T(kopool, [P, TB], "ko", BF16)
                    nc.vector.tensor_add(
                        vo, ps, bv_b[:, n * TB : (n + 1) * TB]
                    )
                    nc.sync.dma_start(
                        v_d[t * P : (t + 1) * P, n * TB : (n + 1) * TB], vo
                    )

            if DEBUG:
                for m in range(KD):
                    nc.sync.dma_start(
                        dbg["d_qt"][m * P : (m + 1) * P, :], qt_all[:, m, :]
                    )
                    nc.sync.dma_start(
                        dbg["d_hq"][m * P : (m + 1) * P, :], hq[:, m, :]
                    )
                    for tb2 in range(NTB):
                        nc.sync.dma_start(
                            dbg["d_h"][
                                m * P : (m + 1) * P, tb2 * TB : (tb2 + 1) * TB
                            ],
                            h_blk[tb2][:, m, :],
                        )
                nc.sync.dma_start(dbg["d_bvb"], bv_b)

        # ================= Stage C: attention + proj =================
        with ExitStack() as sc:
            kpool = sc.enter_context(tc.tile_pool(name="kpool", bufs=3))
            vpool = sc.enter_context(tc.tile_pool(name="vpool", bufs=4))
            ppool = sc.enter_context(tc.tile_pool(name="ppool", bufs=6))
            dpool = sc.enter_context(tc.tile_pool(name="dpool", bufs=4))
            apool = sc.enter_context(tc.tile_pool(name="apool", bufs=1))
            wppool = sc.enter_context(tc.tile_pool(name="wppool", bufs=KD))
            spsum = sc.enter_context(
                tc.tile_pool(name="spsum", bufs=4, space="PSUM")
            )
            apsum = sc.enter_context(
                tc.tile_pool(name="apsum", bufs=4, space="PSUM")
            )

            att_all = T(apool, [P, KD, NQ], "att", BF16)

            # persistent V tiles: the softmax-denominator "ones" columns
            # are written once, only the V slices are re-DMA'd per kt
            vtiles = []
            for i in range(4):
                vt = T(vpool, [P, 4, 2, 64], "v", BF16)
                nc.vector.memset(vt[:, 0::2, 1, :], 1.0)
                nc.vector.memset(vt[:, 1::2, 0, :], 1.0)
                vtiles.append(vt)

            # Wp prefetch tiles; DMAs spread across the hg loop below
            wp_tiles = [
                T(wppool, [P, KD, P], "wp", BF16) for _ in range(KD)
            ]

            # one merged kt loop per head group: for kt < 8 both query
            # chunks are live (N=512 matmuls), beyond that only chunk B
            # (N=256). Head pairs share one 2-bank PSUM tile so exp and
            # mask run as single wide ops. Scores of kt run one step
            # ahead of att@V of kt-1.
            NKT = S // P
            for hg in range(4):
                # trickle the Wp prefetch through the gpsimd queue
                nc.gpsimd.dma_start(wp_tiles[2 * hg], wp[2 * hg])
                nc.gpsimd.dma_start(wp_tiles[2 * hg + 1], wp[2 * hg + 1])
                att_ps = [T(apsum, [P, 2 * QC], "aps") for _ in range(4)]
                kblk = None
                pend = None

                def flush(pend, last):
                    vb, pms, pkt, pc0 = pend
                    for hh in range(4):
                        nc.tensor.matmul(
                            att_ps[hh][:, pc0 : 2 * QC],
                            r(vb[:, hh, :, :]),
                            r(pms[hh]),
                            start=(pkt == 0), stop=last,
                            skip_group_check=True,
                        )

                for kt in range(NKT):
                    c0 = 0 if kt < 8 else QC  # active query columns
                    nq = 2 * QC - c0
                    if kt % 2 == 0:
                        kblk = T(kpool, [P, 2, 2 * P], "k", BF16)
                        nc.sync.dma_start(
                            kblk,
                            kt_d[
                                hg * 256 : (hg + 1) * 256,
                                kt * P : (kt + 2) * P,
                            ].rearrange("(a p) c -> p a c", p=P),
                        )
                    kc = slice((kt % 2) * P, (kt % 2) * P + P)
                    vblk = vtiles[kt % 4]
                    rows = v_d[kt * P : (kt + 1) * P, :]
                    nc.sync.dma_start(
                        vblk[:, 0::2, 0, :],
                        rows[:, hg * 256 : hg * 256 + 192].rearrange(
                            "p (a c) -> p a c", c=64
                        )[:, 0::2, :],
                    )
                    nc.sync.dma_start(
                        vblk[:, 1::2, 1, :],
                        rows[:, hg * 256 + 64 : hg * 256 + 256].rearrange(
                            "p (a c) -> p a c", c=64
                        )[:, 0::2, :],
                    )
                    pms = []
                    thrf = mthr_bf if kt < 8 else mthr2_bf
                    for hh in range(4):
                        b0 = 64 * (hh % 2)
                        sps = T(spsum, [P, 2 * QC], "sps")[:, :nq]
                        nc.tensor.matmul(
                            sps,
                            r(kblk[b0 : b0 + 64, hh // 2, kc]),
                            r(qt_all[
                                b0 : b0 + 64, (hg * 4 + hh) // 2, c0 : 2 * QC
                            ]),
                            start=True, stop=True,
                        )
                        pexp = T(ppool, [P, 2 * QC], "px", BF16)[:, :nq]
                        nc.scalar.activation(pexp, sps, AF.Exp)
                        pm = T(ppool, [P, 2 * QC], "p", BF16)[:, :nq]
                        nc.vector.scalar_tensor_tensor(
                            pm,
                            thrf[:, :nq],
                            kidx_sb[:, kt : kt + 1],
                            pexp,
                            ALU.is_ge,
                            ALU.mult,
                        )
                        pms.append(pm)
                    if pend is not None:
                        flush(pend, last=False)
                    pend = (vblk, pms, kt, c0)
                flush(pend, last=True)

                for hh in range(4):
                    h = hg * 4 + hh
                    base = 64 * (hh % 2)
                    dbase = 64 - base  # rows holding the denominator
                    for qc in range(2):
                        qsl = slice(qc * QC, (qc + 1) * QC)
                        # extract the raw denominator row (plain DVE copy is
                        # partition-safe), move it to physical partition 0,
                        # broadcast, then fast-reciprocal at base 0
                        scr = T(dpool, [P, QC], "scr")
                        nc.vector.tensor_copy(
                            scr[dbase : dbase + 1, :],
                            att_ps[hh][dbase : dbase + 1, qsl],
                        )
                        if dbase != 0:
                            scr0 = T(dpool, [P, QC], "scr")
                            nc.sync.dma_start(
                                scr0[0:1, :], scr[dbase : dbase + 1, :]
                            )
                            src_row = scr0[0:1, :]
                        else:
                            src_row = scr[dbase : dbase + 1, :]
                        db_raw = T(bc, [P, TB], "bcd")[:, :QC]
                        nc.gpsimd.partition_broadcast(db_raw, src_row)
                        rd_b = T(bc, [P, TB], "bc2")[:, :QC]
                        nc.vector.reciprocal_approx_fast(rd_b, db_raw)
                        nc.vector.tensor_mul(
                            att_all[base : base + 64, h // 2, qsl],
                            att_ps[hh][base : base + 64, qsl],
                            rd_b[base : base + 64, :],
                        )

            # proj, fused with bias + residual into res1
            for m in range(KD):
                ps = T(spsum, [P, 2 * QC], "sps")[:, :NQ]
                for kd in range(KD):
                    nc.tensor.matmul(
                        ps, r(wp_tiles[m][:, kd, :]), r(att_all[:, kd, :]),
                        start=(kd == 0), stop=(kd == KD - 1),
                    )
                nc.vector.scalar_tensor_tensor(
                    res1[:, m, :], ps, bp_sb[:, m : m + 1], xq_t[:, m, :],
                    ALU.add, ALU.add,
                )
                nc.scalar.activation(res1b[:, m, :], res1[:, m, :], AF.Copy)
            if DEBUG:
                for m in range(KD):
                    nc.sync.dma_start(
                        dbg["d_att"][m * P : (m + 1) * P, :], att_all[:, m, :]
                    )
                    nc.sync.dma_start(
                        dbg["d_res1"][m * P : (m + 1) * P, :], res1[:, m, :]
                    )
                nc.sync.dma_start(dbg["d_kt"], kt_d)
                nc.sync.dma_start(dbg["d_v"], v_d)

        qs.close()  # free qt/xq SBUF before the FFN

        # ================= Stage D: LN2 + FFN =================
        with ExitStack() as sd:
            mmps = sd.enter_context(
                tc.tile_pool(name="mmpsD", bufs=2, space="PSUM")
            )
            sqpool = sd.enter_context(tc.tile_pool(name="sq2pool", bufs=3))
            h2pool = sd.enter_context(tc.tile_pool(name="h2pool", bufs=1))
            gpool = sd.enter_context(tc.tile_pool(name="gpool", bufs=1))
            partpool = sd.enter_context(tc.tile_pool(name="partpool", bufs=1))
            w1pool = sd.enter_context(tc.tile_pool(name="w1pool", bufs=5))
            w2pool = sd.enter_context(tc.tile_pool(name="w2pool", bufs=3))
            opool = sd.enter_context(tc.tile_pool(name="opool", bufs=3))

            h2 = T(h2pool, [P, KD, NQ], "h2", BF16)
            inv_b, nmi_b = ln_stats(res1b, NQ)
            ln_apply(h2, res1b, inv_b, nmi_b)
            if DEBUG:
                for m in range(KD):
                    nc.sync.dma_start(
                        dbg["d_h2"][m * P : (m + 1) * P, :], h2[:, m, :]
                    )

            # FFN over DFF in two halves; half 0 lands in `part` together
            # with b2 and the residual, half 1 adds on top.
            part = T(partpool, [P, KD, NQ], "part")
            for half in range(2):
                g_h = T(gpool, [P, DHALF, NQ], "g", BF16)
                for mh in range(DHALF):
                    m = half * DHALF + mh
                    wblk = T(w1pool, [P, KD, P], "w1", BF16)
                    nc.gpsimd.dma_start(wblk, w1[m])
                    ps = T(mmps, [P, TB], "mm")[:, :NQ]
                    for kd in range(KD):
                        nc.tensor.matmul(
                            ps, r(wblk[:, kd, :]), r(h2[:, kd, :]),
                            start=(kd == 0), stop=(kd == KD - 1),
                        )
                    nc.scalar.activation(
                        g_h[:, mh, :], ps, AF.Gelu, bias=b1_sb[:, m : m + 1]
                    )

                for m in range(KD):
                    wblk = T(w2pool, [P, DHALF, P], "w2", BF16)
                    nc.gpsimd.dma_start(wblk, w2[half, m])
                    ps = T(mmps, [P, TB], "mm")[:, :NQ]
                    for kd in range(DHALF):
                        nc.tensor.matmul(
                            ps, r(wblk[:, kd, :]), r(g_h[:, kd, :]),
                            start=(kd == 0), stop=(kd == DHALF - 1),
                        )
                    if half == 0:
                        nc.vector.scalar_tensor_tensor(
                            part[:, m, :], ps, b2_sb[:, m : m + 1],
                            res1[:, m, :], ALU.add, ALU.add,
                        )
                    else:
                        ot = T(opool, [P, NQ], "o")
                        nc.vector.tensor_add(ot, ps, part[:, m, :])
                        nc.sync.dma_start(out[m * P : (m + 1) * P, :], ot)

    nc.compile()
    return nc


_NC_CACHE = None


def _get_nc():
    global _NC_CACHE
    if _NC_CACHE is None:
        _NC_CACHE = _build_nc()
    return _NC_CACHE


def _prep_host(inputs):
    """Fold LN affine + score scale into weights; build per-core in_maps."""
    import ml_dtypes

    bf16 = ml_dtypes.bfloat16
    f32 = np.float32
    x = np.ascontiguousarray(np.asarray(inputs["x"], f32))
    gamma = np.asarray(inputs["gamma"], f32)
    beta = np.asarray(inputs["beta"], f32)
    sc = 1.0 / math.sqrt(HS)

    def fold_qkv(W, b, scale):
        W = np.asarray(W, f32)  # [H, D, HS]
        b = np.asarray(b, f32)  # [H, HS]
        Wf = W * gamma[None, :, None]
        bf = b + np.einsum("d,hde->he", beta, W)
        Wf, bf = Wf * scale, bf * scale
        return (
            np.ascontiguousarray(Wf.transpose(1, 0, 2).reshape(D, D)),
            np.ascontiguousarray(bf.reshape(D)),
        )

    wq, bq = fold_qkv(inputs["Wq"], inputs["bq"], sc)
    wk, bk = fold_qkv(inputs["Wk"], inputs["bk"], 1.0)
    wv, bv = fold_qkv(inputs["Wv"], inputs["bv"], 1.0)
    wp = np.ascontiguousarray(np.asarray(inputs["Wp"], f32))
    bp = np.asarray(inputs["bp"], f32)
    W1 = np.asarray(inputs["W1"], f32)
    w1 = np.ascontiguousarray(W1 * gamma[:, None])
    b1 = np.ascontiguousarray(np.asarray(inputs["b1"], f32) + beta @ W1)
    w2 = np.ascontiguousarray(np.asarray(inputs["W2"], f32))
    b2 = np.asarray(inputs["b2"], f32)
    D_, P_, KD_, DH_ = D, P, KD, DHALF

    def blocked(w, din_, dout):
        # [din, dout] -> [dout//P m-blocks, P, din//P kd-blocks, P]:
        # dev[m, p, kd, c] = w[kd*P + p, m*P + c]
        return np.ascontiguousarray(
            w.reshape(din_ // P_, P_, dout // P_, P_)
            .transpose(2, 1, 0, 3)
            .astype(bf16)
        )

    wq = blocked(wq, D_, D_)
    wp = blocked(wp, D_, D_)
    w1 = blocked(w1, D_, DFF)
    # wk resident: dev[p, kd, c] = wk[kd*P + p, c]
    wk = np.ascontiguousarray(
        wk.reshape(KD_, P_, D_).transpose(1, 0, 2).astype(bf16)
    )
    # wv: dev[n, p, kd, c] = wv[kd*P + p, n*TB + c]
    wv = np.ascontiguousarray(
        wv.reshape(KD_, P_, 2, TB).transpose(2, 1, 0, 3).astype(bf16)
    )
    # w2: dev[half, m, p, kd, c] = w2[half*DH_*P + kd*P + p, m*P + c]
    w2 = np.ascontiguousarray(
        w2.reshape(2, DH_, P_, KD_, P_).transpose(0, 3, 2, 1, 4).astype(bf16)
    )

    kidx = (
        np.arange(P)[:, None] + P * np.arange(S // P)[None, :]
    ).astype(np.float16)

    in_maps = []
    for core in range(N_CORES):
        b, i = divmod(core, 4)
        xt = np.ascontiguousarray(x[b].T)  # [D, S]
        ca = slice(QC * i, QC * (i + 1))
        cb = slice(QC * (7 - i), QC * (8 - i))
        xq = np.ascontiguousarray(np.concatenate([xt[:, ca], xt[:, cb]], 1))
        thrA = np.arange(ca.start, ca.stop, dtype=f32)
        thrB = np.arange(cb.start, cb.stop, dtype=f32)
        ab = np.concatenate([thrA, thrB])
        bb = np.concatenate([thrB, thrB])
        mthr = np.ascontiguousarray(np.stack([ab, ab])[None].astype(f32))
        mthr2 = np.ascontiguousarray(np.stack([bb, bb])[None].astype(f32))
        in_maps.append(
            dict(
                xt=np.ascontiguousarray(xt.astype(bf16)),
                xq=xq,
                xqb=np.ascontiguousarray(xq.astype(bf16)),
                wq=wq, wk=wk, wv=wv, wp=wp, w1=w1, w2=w2,
                bq=bq, bk=bk, bv=bv, bp=bp, b1=b1, b2=b2,
                mthr=mthr, mthr2=mthr2, kidx=kidx,
            )
        )
    return in_maps


def _assemble(results):
    y = np.empty((B, S, D), np.float32)
    for core in range(N_CORES):
        b, i = divmod(core, 4)
        o = results[core]["out"]  # [D, NQ]
        y[b, QC * i : QC * (i + 1), :] = o[:, :QC].T
        y[b, QC * (7 - i) : QC * (8 - i), :] = o[:, QC:].T
    return y


def kernel(**inputs):
    from concourse.bass_utils import run_bass_kernel_spmd

    nc = _get_nc()
    in_maps = _prep_host(inputs)
    res = run_bass_kernel_spmd(nc, in_maps, list(range(N_CORES)))
    return _assemble(res.results)

